# revision 1
# baseline (speedup 1.0000x reference)
"""nn_BlockCirculantLinear on 8 Trainium2 cores (Bass/Tile, float32r).

Math.  The reference computes, per output block o (8 blocks of P=512):
    y_o = sum_i real(IFFT(Lam[o,i] * FFT(x_i * sf_i)))
With x real, this factors exactly into three real linear stages:
  1. forward  : X_i = Fe @ (sf*x)_i^T      -- real-DFT coords, per block i
  2. middle   : Y_o = sum_i M_oi X_i       -- per-frequency 2x2 mixes
  3. inverse  : y_o^T = Fi @ Y_o
Coordinate packing per block: c=0 -> (f=0, re); c=1 -> (f=256, re);
c=2f/2f+1 -> (f, re/im) for f=1..255.  Frequency pair (f, P-f) is folded
into one 2x2 real block using the Hermitian symmetry of X:
  A_f = (l1r+l2r) Xr + (l2i-l1i) Xi ;  B_f = (l1i-l2i) Xr + (l1r+l2r) Xi
where l1 = Lam[o,i,f], l2 = Lam[o,i,P-f], and
  y[t] = (1/P)[A_0 + A_256 (-1)^t + sum_f (A_f cos(2pi f t/P) - B_f sin(..))].
This does 34 GFLOP/core of 128x128-tile matmuls (half of the dense-W
formulation) with only ~18 MiB of transform constants.

Sharding: data-parallel -- 16384 rows split 8 ways; constants replicated.
sign_flip is folded into x on the host; bias is added on the host after
gathering (host also transposes x in / y^T out, which is free input/output
marshalling).

Device kernel (per core): 2048 float32r matmuls of [K=128, M=128, N=512],
PSUM-resident accumulation (fwd K=512 in 4; mid sum over 8 blocks; inv K=512
in 4), psum pools 2/4/2 banks, mid constants streamed with 9-deep prefetch,
outputs evicted via DVE/ACT copies.  Measured ~415 us/core/pass on HW.
"""
import os
from contextlib import ExitStack

import numpy as np

import concourse.mybir as mybir
import concourse.bacc as bacc
import concourse.tile as tile
from concourse.bass_utils import run_bass_kernel_spmd

N_CORES = 8
ROWS = 16384
RPC = ROWS // N_CORES      # 2048 rows per core
F = 4096
P = 512
NBLK = 8
CHUNK = 512                # rows per pipelined chunk (= matmul free dim)
_NC_CACHE = {}

DT = mybir.dt.float32r     # fp32 in memory, FP22 in the PE, fp32 accumulate
DTO = mybir.dt.float32


def build_transforms(spectral_real, spectral_imag, dtype=np.float64):
    """Fe [c, feat], Fi [t, c], M [o, i, c_out, c_in] (2x2 block diagonal)."""
    s = np.arange(P)
    f = np.arange(1, P // 2)
    theta = 2 * np.pi * np.outer(f, s) / P

    Fe = np.zeros((P, P), dtype)
    Fe[0, :] = 1.0
    Fe[1, :] = (-1.0) ** s
    Fe[2::2, :] = np.cos(theta)
    Fe[3::2, :] = -np.sin(theta)

    Fi = np.zeros((P, P), dtype)
    Fi[:, 0] = 1.0 / P
    Fi[:, 1] = ((-1.0) ** s) / P
    Fi[:, 2::2] = np.cos(theta).T / P
    Fi[:, 3::2] = -np.sin(theta).T / P

    lam_r = spectral_real.astype(dtype)
    lam_i = spectral_imag.astype(dtype)
    M = np.zeros((NBLK, NBLK, P, P), dtype)
    M[:, :, 0, 0] = lam_r[:, :, 0]
    M[:, :, 1, 1] = lam_r[:, :, P // 2]
    l1r = lam_r[:, :, 1:P // 2]; l1i = lam_i[:, :, 1:P // 2]
    l2r = lam_r[:, :, :P // 2:-1]; l2i = lam_i[:, :, :P // 2:-1]
    ce = np.arange(2, P, 2); co = ce + 1
    M[:, :, ce, ce] = l1r + l2r
    M[:, :, ce, co] = l2i - l1i
    M[:, :, co, ce] = l1i - l2i
    M[:, :, co, co] = l1r + l2r
    return Fe, Fi, M


def host_transforms(spectral_real, spectral_imag):
    Fe, Fi, M = build_transforms(spectral_real, spectral_imag)
    fwdT = np.ascontiguousarray(Fe.T.astype(np.float32))     # lhsT [feat, c]
    invT = np.ascontiguousarray(Fi.T.astype(np.float32))     # lhsT [c, t]
    # mid lhsT tiles packed per (o, ct): [128, 8 blocks * 128]
    midT = np.zeros((NBLK, 4, 128, NBLK * 128), np.float32)
    for o in range(NBLK):
        for ct in range(4):
            sl = slice(ct * 128, (ct + 1) * 128)
            for i in range(NBLK):
                midT[o, ct, :, i * 128:(i + 1) * 128] = M[o, i, sl, sl].T
    return fwdT, invT, midT


def build_nc(repeat: int = 1):
    key = (CHUNK, repeat)
    if key in _NC_CACHE:
        return _NC_CACHE[key]
    nc = bacc.Bacc("TRN2", target_bir_lowering=False, debug=False,
                   num_devices=N_CORES)
    xT = nc.dram_tensor("xT", [F, RPC], DT, kind="ExternalInput")
    fwdT = nc.dram_tensor("fwdT", [P, P], DT, kind="ExternalInput")
    invT = nc.dram_tensor("invT", [P, P], DT, kind="ExternalInput")
    midT = nc.dram_tensor("midT", [NBLK, 4, 128, NBLK * 128], DT,
                          kind="ExternalInput")
    yT = nc.dram_tensor("yT", [F, RPC], DTO, kind="ExternalOutput")

    n_chunks = RPC // CHUNK

    with tile.TileContext(nc) as tc:
        with ExitStack() as ctx:
            const = ctx.enter_context(tc.tile_pool(name="const", bufs=1))
            fwd_sb = const.tile([128, 16 * 128], DT)
            inv_sb = const.tile([128, 16 * 128], DT)
            for kc in range(4):
                for mt in range(4):
                    j = (kc * 4 + mt) * 128
                    nc.sync.dma_start(fwd_sb[:, j:j + 128],
                                      fwdT[kc * 128:(kc + 1) * 128,
                                           mt * 128:(mt + 1) * 128])
                    nc.sync.dma_start(inv_sb[:, j:j + 128],
                                      invT[kc * 128:(kc + 1) * 128,
                                           mt * 128:(mt + 1) * 128])

            xpool = ctx.enter_context(tc.tile_pool(name="x", bufs=16))
            Xpool = ctx.enter_context(tc.tile_pool(name="X", bufs=34))
            Ypool = ctx.enter_context(tc.tile_pool(name="Y", bufs=10))
            mpool = ctx.enter_context(tc.tile_pool(name="mid", bufs=9))
            opool = ctx.enter_context(tc.tile_pool(name="out", bufs=5))
            psf = ctx.enter_context(tc.tile_pool(name="psf", bufs=2,
                                                 space="PSUM"))
            psm = ctx.enter_context(tc.tile_pool(name="psm", bufs=4,
                                                 space="PSUM"))
            psi = ctx.enter_context(tc.tile_pool(name="psi", bufs=2,
                                                 space="PSUM"))

            def chunk_body(c):
                r0 = c * CHUNK
                x_sb = {}
                for i in range(NBLK):
                    for kc in range(4):
                        t = xpool.tile([128, CHUNK], DT, tag="x", name="xt")
                        nc.sync.dma_start(
                            t[:], xT[(i * 4 + kc) * 128:(i * 4 + kc + 1) * 128,
                                     r0:r0 + CHUNK])
                        x_sb[i, kc] = t
                X_sb = {}
                for i in range(NBLK):
                    for mt in range(4):
                        ps = psf.tile([128, CHUNK], DTO, tag="f", name="fps")
                        for kc in range(4):
                            nc.tensor.matmul(
                                ps[:],
                                fwd_sb[:, (kc * 4 + mt) * 128:(kc * 4 + mt + 1) * 128],
                                x_sb[i, kc][:],
                                start=(kc == 0), stop=(kc == 3))
                        t = Xpool.tile([128, CHUNK], DT, tag="X", name="Xt")
                        nc.any.tensor_copy(out=t[:], in_=ps[:])
                        X_sb[i, mt] = t
                for o in range(NBLK):
                    Y_sb = {}
                    for ct in range(4):
                        m = mpool.tile([128, NBLK * 128], DT, tag="m",
                                       name="mt_")
                        nc.sync.dma_start(m[:], midT[o, ct])
                        ps = psm.tile([128, CHUNK], DTO, tag="m", name="mps")
                        for i in range(NBLK):
                            nc.tensor.matmul(
                                ps[:], m[:, i * 128:(i + 1) * 128],
                                X_sb[i, ct][:],
                                start=(i == 0), stop=(i == NBLK - 1))
                        t = Ypool.tile([128, CHUNK], DT, tag="Y", name="Yt")
                        nc.any.tensor_copy(out=t[:], in_=ps[:])
                        Y_sb[ct] = t
                    for tt in range(4):
                        ps = psi.tile([128, CHUNK], DTO, tag="i", name="ips")
                        for ct in range(4):
                            nc.tensor.matmul(
                                ps[:],
                                inv_sb[:, (ct * 4 + tt) * 128:(ct * 4 + tt + 1) * 128],
                                Y_sb[ct][:],
                                start=(ct == 0), stop=(ct == 3))
                        t = opool.tile([128, CHUNK], DTO, tag="o", name="ot")
                        nc.any.tensor_copy(out=t[:], in_=ps[:])
                        nc.sync.dma_start(
                            yT[(o * 4 + tt) * 128:(o * 4 + tt + 1) * 128,
                               r0:r0 + CHUNK], t[:])

            def body(_=None):
                for c in range(n_chunks):
                    chunk_body(c)

            if repeat == 1:
                body()
            else:
                with tc.For_i(0, repeat, 1) as it:
                    body(it)
    nc.compile()
    _NC_CACHE[key] = nc
    return nc


def make_in_maps(x, spectral_real, spectral_imag, sign_flip):
    fwdT, invT, midT = host_transforms(spectral_real, spectral_imag)
    xs = (x.reshape(-1, F) * sign_flip[None, :].astype(np.float32))
    in_maps = []
    for c in range(N_CORES):
        shard = xs[c * RPC:(c + 1) * RPC]
        in_maps.append({
            "xT": np.ascontiguousarray(shard.T),
            "fwdT": fwdT, "invT": invT, "midT": midT,
        })
    return in_maps


def kernel(x, spectral_real, spectral_imag, sign_flip, bias):
    x = np.asarray(x, np.float32)
    spectral_real = np.asarray(spectral_real, np.float32)
    spectral_imag = np.asarray(spectral_imag, np.float32)
    sign_flip = np.asarray(sign_flip, np.float32)
    bias = np.asarray(bias, np.float32)
    batch_shape = x.shape[:-1]

    in_maps = make_in_maps(x, spectral_real, spectral_imag, sign_flip)
    nc = build_nc()
    res = run_bass_kernel_spmd(nc, in_maps, list(range(N_CORES)))
    y = np.concatenate(
        [np.ascontiguousarray(res.results[c]["yT"].T) for c in range(N_CORES)],
        axis=0)
    y = y + bias[None, :]
    return y.reshape(*batch_shape, F).astype(np.float32)



# revision 2
# speedup vs baseline: 1.3538x; 1.3538x over previous
"""nn_BlockCirculantLinear on 8 Trainium2 cores (Bass/Tile, bf16, packed mid).

Math.  Per output block o (8 blocks of P=512):
    y_o = sum_i real(IFFT(Lam[o,i] * FFT(x_i * sf_i)))
factors into three real linear stages (real-DFT coordinates, frequency
pair (f, P-f) folded into a 2x2 real block):
  1. forward  : X_i = Fe @ (sf*x)_i^T      -- dense 512x512 per block i
  2. middle   : Y_o = sum_i M_oi X_i       -- per-frequency-pair 2x2 mixes
  3. inverse  : y_o^T = Fi @ Y_o           -- dense 512x512 per block o

The middle stage is 2x2-block-diagonal (1.5% dense), so instead of dense
128x128 tiles over (o,i) [256 matmuls/chunk] we re-pack partitions so one
matmul covers 8 frequency pairs for ALL (i -> o) at once [32 matmuls/chunk]:
  packed tile T=(ct,g): partition p = i*16 + q  holds coord ct*128+g*16+q
  of block i; lhsT[T][i*16+q_in, o*16+q_out] = M[o,i,C+q_out,C+q_in].
The repack is a partition-slab DMA shuffle.  To keep every shuffle DMA a
single-partition-dim 3-dim AP, forward outputs are produced PRE-PERMUTED
(per 128-coord tile, PSUM partition p holds coord (p%8)*16 + p//8, folded
into Fe's row order); the inverse's K-dim uses the same permuted order
(folded into Fi's column order).  Zero-cost on device.

Per chunk of 512 rows: 128 fwd + 32 mid + 128 inv matmuls of
[K=128,M=128,N=512] bf16 (213ns each) = 61us PE time; DMA: x-in 8, Xshuf
32 (Pool/SWDGE), Yshuf 32 (Act/HWDGE), y-out 8 (SP/HWDGE) = 80 DMAs,
16.8MB.  Stages are software-pipelined across chunks (mid/inv of chunk
c-1 interleave with fwd of chunk c) so the PE queue never drains.

Sharding: data-parallel -- 16384 rows split 8 ways; constants replicated.
sign_flip folded into x on host; bias added on host after gathering.
"""
import os
from contextlib import ExitStack

import numpy as np
import ml_dtypes

import concourse.mybir as mybir
import concourse.bacc as bacc
import concourse.tile as tile
from concourse.bass_utils import run_bass_kernel_spmd

N_CORES = 8
ROWS = 16384
RPC = ROWS // N_CORES      # 2048 rows per core
F = 4096
P = 512
NBLK = 8
CHUNK = 512                # rows per pipelined chunk (= matmul free dim)
NCT = 4                    # 128-coord tiles per block
_NC_CACHE = {}

DT = mybir.dt.bfloat16
DTO = mybir.dt.float32

# partition p of a forward-output tile holds local coord PERM[p]
PERM = np.array([(p % 8) * 16 + p // 8 for p in range(128)])


def build_transforms(spectral_real, spectral_imag, dtype=np.float64):
    """Fe [c, feat], Fi [t, c], M [o, i, c_out, c_in] (2x2 block diagonal)."""
    s = np.arange(P)
    f = np.arange(1, P // 2)
    theta = 2 * np.pi * np.outer(f, s) / P

    Fe = np.zeros((P, P), dtype)
    Fe[0, :] = 1.0
    Fe[1, :] = (-1.0) ** s
    Fe[2::2, :] = np.cos(theta)
    Fe[3::2, :] = -np.sin(theta)

    Fi = np.zeros((P, P), dtype)
    Fi[:, 0] = 1.0 / P
    Fi[:, 1] = ((-1.0) ** s) / P
    Fi[:, 2::2] = np.cos(theta).T / P
    Fi[:, 3::2] = -np.sin(theta).T / P

    lam_r = spectral_real.astype(dtype)
    lam_i = spectral_imag.astype(dtype)
    M = np.zeros((NBLK, NBLK, P, P), dtype)
    M[:, :, 0, 0] = lam_r[:, :, 0]
    M[:, :, 1, 1] = lam_r[:, :, P // 2]
    l1r = lam_r[:, :, 1:P // 2]; l1i = lam_i[:, :, 1:P // 2]
    l2r = lam_r[:, :, :P // 2:-1]; l2i = lam_i[:, :, :P // 2:-1]
    ce = np.arange(2, P, 2); co = ce + 1
    M[:, :, ce, ce] = l1r + l2r
    M[:, :, ce, co] = l2i - l1i
    M[:, :, co, ce] = l1i - l2i
    M[:, :, co, co] = l1r + l2r
    return Fe, Fi, M


def host_transforms(spectral_real, spectral_imag):
    Fe, Fi, M = build_transforms(spectral_real, spectral_imag)
    # fwd lhsT [k, (kc*4+mt)*128 + p] = Fe[mt*128 + PERM[p], kc*128 + k]
    fwdT = np.zeros((128, 16 * 128), np.float32)
    for kc in range(4):
        for mt in range(4):
            blk = Fe[mt * 128:(mt + 1) * 128, kc * 128:(kc + 1) * 128]
            fwdT[:, (kc * 4 + mt) * 128:(kc * 4 + mt + 1) * 128] = \
                blk[PERM, :].T
    # inv lhsT [p, (ct*4+tt)*128 + t] = Fi[tt*128 + t, ct*128 + PERM[p]]
    invT = np.zeros((128, 16 * 128), np.float32)
    for ct in range(4):
        for tt in range(4):
            blk = Fi[tt * 128:(tt + 1) * 128, ct * 128:(ct + 1) * 128]
            invT[:, (ct * 4 + tt) * 128:(ct * 4 + tt + 1) * 128] = \
                blk[:, PERM].T
    # mid lhsT [i*16+q_in, T*128 + o*16+q_out] = M[o,i,C+q_out,C+q_in]
    midT = np.zeros((128, 32 * 128), np.float32)
    for ct in range(4):
        for g in range(8):
            T = ct * 8 + g
            C = ct * 128 + g * 16
            for o in range(NBLK):
                for i in range(NBLK):
                    midT[i * 16:(i + 1) * 16,
                         T * 128 + o * 16:T * 128 + (o + 1) * 16] = \
                        M[o, i, C:C + 16, C:C + 16].T
    bf = ml_dtypes.bfloat16
    return fwdT.astype(bf), invT.astype(bf), midT.astype(bf)


def build_nc(repeat: int = 1):
    key = (CHUNK, repeat)
    if key in _NC_CACHE:
        return _NC_CACHE[key]
    nc = bacc.Bacc("TRN2", target_bir_lowering=False, debug=False,
                   num_devices=N_CORES)
    xT = nc.dram_tensor("xT", [F, RPC], DT, kind="ExternalInput")
    fwdT = nc.dram_tensor("fwdT", [128, 16 * 128], DT, kind="ExternalInput")
    invT = nc.dram_tensor("invT", [128, 16 * 128], DT, kind="ExternalInput")
    midT = nc.dram_tensor("midT", [128, 32 * 128], DT, kind="ExternalInput")
    yT = nc.dram_tensor("yT", [F, RPC], DT, kind="ExternalOutput")

    n_chunks = RPC // CHUNK

    with tile.TileContext(nc) as tc:
        with ExitStack() as ctx:
            const = ctx.enter_context(tc.tile_pool(name="const", bufs=1))
            fwd_sb = const.tile([128, 16 * 128], DT)
            inv_sb = const.tile([128, 16 * 128], DT)
            mid_sb = const.tile([128, 32 * 128], DT)
            nc.sync.dma_start(fwd_sb[:], fwdT[:])
            nc.sync.dma_start(inv_sb[:], invT[:])
            nc.sync.dma_start(mid_sb[:], midT[:])

            xpool = ctx.enter_context(tc.tile_pool(name="x", bufs=10))
            Xpool = ctx.enter_context(tc.tile_pool(name="X", bufs=34))
            pkpool = ctx.enter_context(tc.tile_pool(name="pk", bufs=3))
            pypool = ctx.enter_context(tc.tile_pool(name="py", bufs=3))
            Ypool = ctx.enter_context(tc.tile_pool(name="Y", bufs=34))
            opool = ctx.enter_context(tc.tile_pool(name="out", bufs=4))
            psf = ctx.enter_context(tc.tile_pool(name="psf", bufs=3,
                                                 space="PSUM"))
            psm = ctx.enter_context(tc.tile_pool(name="psm", bufs=2,
                                                 space="PSUM"))
            psi = ctx.enter_context(tc.tile_pool(name="psi", bufs=3,
                                                 space="PSUM"))

            def emit_loads(c):
                r0 = c * CHUNK
                xb = {}
                for i in range(NBLK):
                    t = xpool.tile([128, 4 * CHUNK], DT, tag="x", name="xt")
                    nc.sync.dma_start(
                        t.rearrange("p (kc col) -> p kc col", kc=4),
                        xT[i * 512:(i + 1) * 512, r0:r0 + CHUNK].rearrange(
                            "(kc p) col -> p kc col", kc=4))
                    xb[i] = t
                return xb

            def emit_fwd(xb):
                """fwd matmuls + evict + Xshuf; returns pk tiles per ct."""
                pk = {ct: pkpool.tile([128, 8 * CHUNK], DT, tag="pk",
                                      name="pkt") for ct in range(NCT)}
                for i in range(NBLK):
                    for mt in range(NCT):
                        ps = psf.tile([128, CHUNK], DTO, tag="f", name="fps")
                        for kc in range(4):
                            j = (kc * 4 + mt) * 128
                            nc.tensor.matmul(
                                ps[:], fwd_sb[:, j:j + 128],
                                xb[i][:, kc * CHUNK:(kc + 1) * CHUNK],
                                start=(kc == 0), stop=(kc == 3))
                        t = Xpool.tile([128, CHUNK], DT, tag="X", name="Xt")
                        nc.any.tensor_copy(out=t[:], in_=ps[:])
                        nc.gpsimd.dma_start(
                            pk[mt][i * 16:(i + 1) * 16, :].rearrange(
                                "q (g col) -> q g col", g=8),
                            t[:])
                return pk

            def emit_mid(pk):
                """packed mid matmuls + evict + Yshuf; returns Y tiles."""
                Yt = {}
                for ct in range(NCT):
                    py = pypool.tile([128, 8 * CHUNK], DT, tag="py",
                                     name="pyt")
                    for g in range(8):
                        T = ct * 8 + g
                        ps = psm.tile([128, CHUNK], DTO, tag="m", name="mps")
                        nc.tensor.matmul(
                            ps[:], mid_sb[:, T * 128:(T + 1) * 128],
                            pk[ct][:, g * CHUNK:(g + 1) * CHUNK],
                            start=True, stop=True)
                        nc.any.tensor_copy(
                            out=py[:, g * CHUNK:(g + 1) * CHUNK], in_=ps[:])
                    for o in range(NBLK):
                        t = Ypool.tile([128, CHUNK], DT, tag="Y", name="Yt")
                        nc.scalar.dma_start(
                            t[:],
                            py[o * 16:(o + 1) * 16, :].rearrange(
                                "q (g col) -> q g col", g=8))
                        Yt[o, ct] = t
                return Yt

            def emit_inv(Yt, c):
                r0 = c * CHUNK
                for o in range(NBLK):
                    ob = opool.tile([128, 4 * CHUNK], DT, tag="o", name="ot")
                    for tt in range(4):
                        ps = psi.tile([128, CHUNK], DTO, tag="i", name="ips")
                        for ct in range(4):
                            j = (ct * 4 + tt) * 128
                            nc.tensor.matmul(
                                ps[:], inv_sb[:, j:j + 128], Yt[o, ct][:],
                                start=(ct == 0), stop=(ct == 3))
                        nc.any.tensor_copy(
                            out=ob[:, tt * CHUNK:(tt + 1) * CHUNK], in_=ps[:])
                    nc.sync.dma_start(
                        yT[o * 512:(o + 1) * 512, r0:r0 + CHUNK].rearrange(
                            "(tt p) col -> p tt col", tt=4),
                        ob.rearrange("p (tt col) -> p tt col", tt=4))

            def body(_=None):
                # software pipeline: mid/inv of chunk c-1 interleave with
                # fwd of chunk c on the PE queue.
                prev_pk = None
                prev_c = None
                for c in range(n_chunks):
                    xb = emit_loads(c)
                    if prev_pk is not None:
                        Yt = emit_mid(prev_pk)
                    pk = emit_fwd(xb)
                    if prev_pk is not None:
                        emit_inv(Yt, prev_c)
                    prev_pk, prev_c = pk, c
                Yt = emit_mid(prev_pk)
                emit_inv(Yt, prev_c)

            if repeat == 1:
                body()
            else:
                with tc.For_i(0, repeat, 1) as it:
                    body(it)
    nc.compile()
    _NC_CACHE[key] = nc
    return nc


def make_in_maps(x, spectral_real, spectral_imag, sign_flip):
    fwdT, invT, midT = host_transforms(spectral_real, spectral_imag)
    bf = ml_dtypes.bfloat16
    xs = (x.reshape(-1, F) * sign_flip[None, :].astype(np.float32))
    in_maps = []
    for c in range(N_CORES):
        shard = xs[c * RPC:(c + 1) * RPC]
        in_maps.append({
            "xT": np.ascontiguousarray(shard.T).astype(bf),
            "fwdT": fwdT, "invT": invT, "midT": midT,
        })
    return in_maps


def kernel(x, spectral_real, spectral_imag, sign_flip, bias):
    x = np.asarray(x, np.float32)
    spectral_real = np.asarray(spectral_real, np.float32)
    spectral_imag = np.asarray(spectral_imag, np.float32)
    sign_flip = np.asarray(sign_flip, np.float32)
    bias = np.asarray(bias, np.float32)
    batch_shape = x.shape[:-1]

    in_maps = make_in_maps(x, spectral_real, spectral_imag, sign_flip)
    nc = build_nc()
    res = run_bass_kernel_spmd(nc, in_maps, list(range(N_CORES)))
    y = np.concatenate(
        [np.ascontiguousarray(
            np.asarray(res.results[c]["yT"], np.float32).T)
         for c in range(N_CORES)],
        axis=0)
    y = y + bias[None, :]
    return y.reshape(*batch_shape, F).astype(np.float32)
